# revision 11
# baseline (speedup 1.0000x reference)
"""Trainium2 Bass kernel for nn_BCHConv2D (complex harmonic conv + bispectrum).

Strategy (8 NeuronCores, data-parallel over batch B=8):
  host: build complex-harmonic filters from w+atoms -> fd [7,224,NPAD],
        transpose each batch image to (H, C, W), replicate bias.
  core: 7x7x32 -> 224ch conv as 14 accumulating matmuls per output row
        (contraction (kw,c) split 128+96; stationary = im2row stack DMA'd
        straight from DRAM with overlapping APs; moving = filter chunk).
        PSUM [122,NPAD] (spatial on partitions) -> fp16 SBUF -> bispectrum
        as blocked elementwise ops (DVE/ACT/GPSIMD) -> bias+relu -> out.
"""
import os
import sys
import types
from itertools import product

import numpy as np

sys.path.insert(0, "/opt/trn_rl_repo")
sys.path.insert(0, "/root/.axon_site")

import concourse.bass as bass
import concourse.bacc as bacc
import concourse.tile as tile
from concourse import mybir
from concourse import bass_utils

# ---------------- problem constants ----------------
KS, MD, STREAMS, C_IN = 7, 6, 16, 32
H = W = 128
HO = WO = 122
NC_RE = (MD + 1) * STREAMS       # 112
NCONV = 2 * NC_RE                # 224
NB = 8                           # batch == cores
PAIRS = [(1, 1), (1, 2), (1, 3), (2, 2), (2, 3), (3, 3)]

# ---------------- tuning knobs ----------------
CONV_DT = os.environ.get("CONV_DT", "f32r")      # f32r | f16 | bf16
NPAD = int(os.environ.get("NPAD", "256" if CONV_DT == "f32r" else "224"))
R = int(os.environ.get("BISP_R", "16"))          # rows per bispectrum group

F16 = mybir.dt.float16
_DT_MAP = {"f32r": mybir.dt.float32r, "f16": mybir.dt.float16,
           "bf16": mybir.dt.bfloat16}
_NP_MAP = {"f32r": np.float32, "f16": np.float16, "bf16": None}  # bf16 via ml_dtypes


def _np_conv_dtype():
    if CONV_DT == "bf16":
        import ml_dtypes
        return ml_dtypes.bfloat16
    return _NP_MAP[CONV_DT]


# ---------------- host-side filter construction ----------------
def _tri(v):
    return np.where(np.abs(v) <= 1, np.where(v < 0, v + 1, 1 - v), 0)


def _make_atoms(kernel_size, max_degree):
    radius = (kernel_size - 1) // 2
    g = np.arange(-radius, radius + 1)
    xg, yg = np.meshgrid(g, g)
    r = np.sqrt(xg ** 2 + yg ** 2)
    theta = np.arctan2(yg, xg)
    n_rp = kernel_size // 2 + 1
    atoms = np.zeros((kernel_size, kernel_size, max_degree + 1, n_rp),
                     dtype=np.complex64)
    for i, n in product(range(n_rp), range(max_degree + 1)):
        atoms[:, :, n, i] = _tri(r - i) * np.exp(theta * n * -1j)
    atoms[kernel_size // 2, kernel_size // 2, 1:, :] = 0
    norm = np.sqrt(np.sum(np.conj(atoms) * atoms, axis=(0, 1)))
    norm[norm == 0] = 1
    return (atoms / norm).astype(np.complex64)


_ATOMS = _make_atoms(KS, MD)


def _host_filters(w):
    """w (1,1,32,16,7,4) -> fd [7, 224, NPAD] float32.
    Conv channel order: col n*16+s = re(n,s); 112+n*16+s = im(n,s)."""
    wc = w[0, 0]
    f_re = np.einsum("hwnr,csnr->hwcsn", _ATOMS.real, wc)
    f_im = np.einsum("hwnr,csnr->hwcsn", _ATOMS.imag, wc)
    filt = np.zeros((KS, KS, C_IN, NPAD), np.float32)
    filt[:, :, :, 0:NC_RE] = np.transpose(f_re, (0, 1, 2, 4, 3)).reshape(
        KS, KS, C_IN, NC_RE)
    filt[:, :, :, NC_RE:NCONV] = np.transpose(f_im, (0, 1, 2, 4, 3)).reshape(
        KS, KS, C_IN, NC_RE)
    fd = np.zeros((KS, 224, NPAD), np.float32)
    fd[:, 0:128, :] = filt[:, 0:4].reshape(KS, 128, NPAD)
    fd[:, 128:224, :] = filt[:, 4:7].reshape(KS, 96, NPAD)
    return fd


# ---------------- bass program ----------------
def _ap(src_ap, off, dims):
    """New AP into the same tensor: explicit [step, count] dims (elements)."""
    return bass.AP(tensor=src_ap.tensor, offset=src_ap.offset + off, ap=dims)


_PROGRAM = None


def _build_program():
    cdt = _DT_MAP[CONV_DT]
    nc = bacc.Bacc("TRN2", target_bir_lowering=False, debug=False,
                   num_devices=NB)
    xt_d = nc.dram_tensor("xt", [H, C_IN, W], cdt, kind="ExternalInput").ap()
    filt_d = nc.dram_tensor("filt", [KS, 224, NPAD], cdt,
                            kind="ExternalInput").ap()
    bias_d = nc.dram_tensor("biasrep", [128, R * 256], F16,
                            kind="ExternalInput").ap()
    out_d = nc.dram_tensor("out", [HO, WO, 256], F16,
                           kind="ExternalOutput").ap()
    MUL = mybir.AluOpType.mult
    ADD = mybir.AluOpType.add
    SUB = mybir.AluOpType.subtract

    with tile.TileContext(nc) as tc:
        with tc.tile_pool(name="const", bufs=1) as constp, \
             tc.tile_pool(name="stk", bufs=4) as stkp, \
             tc.tile_pool(name="fm", bufs=4) as fmp, \
             tc.tile_pool(name="blk", bufs=2) as blkp, \
             tc.tile_pool(name="tmp", bufs=2) as tmpp, \
             tc.tile_pool(name="yp", bufs=3) as yp, \
             tc.tile_pool(name="ofp", bufs=2) as ofp, \
             tc.tile_pool(name="ps", bufs=8, space="PSUM") as psp:

            # ---- constants ----
            fA = constp.tile([128, KS * NPAD], cdt)
            fB = constp.tile([96, KS * NPAD], cdt)
            for kh in range(KS):
                nc.scalar.dma_start(fA[:, kh * NPAD:(kh + 1) * NPAD],
                                    filt_d[kh, 0:128, :])
                nc.scalar.dma_start(fB[:, kh * NPAD:(kh + 1) * NPAD],
                                    filt_d[kh, 128:224, :])
            biasT = constp.tile([128, R * 256], F16)
            nc.scalar.dma_start(biasT[:], bias_d[:])

            psum_by_h = {}
            group = {}   # current bispectrum group state

            def sview(t, off, dims, nparts=WO):
                a = t[:]
                return bass.AP(tensor=a.tensor, offset=a.offset + off,
                               ap=[[a.ap[0][0], nparts]] + dims)

            def bisp(fmT, h0, Rg):
                """Emit bispectrum ops for rows h0..h0+Rg-1 held in fmT."""
                def fmv(off, dims):
                    return sview(fmT, off, [[NCONV, Rg]] + dims)

                # gather blocks [122, Rg*96] fp16, pair-major per row
                Ar = blkp.tile([128, R * 96], F16, tag="Ar")
                Ai = blkp.tile([128, R * 96], F16, tag="Ai")
                Br = blkp.tile([128, R * 96], F16, tag="Br")
                Bi = blkp.tile([128, R * 96], F16, tag="Bi")
                Cr = blkp.tile([128, R * 96], F16, tag="Cr")
                Ci = blkp.tile([128, R * 96], F16, tag="Ci")
                # copies: (n_start, pair_start, count)
                gA = [(1, 0, 1), (1, 1, 1), (1, 2, 2), (2, 4, 2)]
                gB = [(1, 0, 3), (2, 3, 2), (3, 5, 1)]
                gC = [(2, 0, 3), (4, 3, 2), (6, 5, 1)]
                for (blk_r, blk_i, runs, eng) in (
                        (Ar, Ai, gA, nc.scalar.copy),
                        (Br, Bi, gB, nc.scalar.copy),
                        (Cr, Ci, gC, nc.vector.tensor_copy)):
                    for (n0, p0, cnt) in runs:
                        L = cnt * 16
                        eng(sview(blk_r, p0 * 16, [[96, Rg], [1, L]]),
                            fmv(n0 * 16, [[1, L]]))
                        eng(sview(blk_i, p0 * 16, [[96, Rg], [1, L]]),
                            fmv(NC_RE + n0 * 16, [[1, L]]))

                t1 = tmpp.tile([128, R * 96], F16, tag="t1")
                t2 = tmpp.tile([128, R * 96], F16, tag="t2")
                t3 = tmpp.tile([128, R * 96], F16, tag="t3")
                t4 = tmpp.tile([128, R * 96], F16, tag="t4")
                full = lambda t: t[0:WO, 0:Rg * 96]
                # stage 1: re1 = ArBr - AiBi (t1), im1 = ArBi + AiBr (t3)
                nc.vector.tensor_tensor(full(t1), full(Ar), full(Br), MUL)
                nc.vector.tensor_tensor(full(t2), full(Ai), full(Bi), MUL)
                nc.vector.tensor_tensor(full(t1), full(t1), full(t2), SUB)
                nc.vector.tensor_tensor(full(t3), full(Ar), full(Bi), MUL)
                nc.vector.tensor_tensor(full(t4), full(Ai), full(Br), MUL)
                nc.vector.tensor_tensor(full(t3), full(t3), full(t4), ADD)
                # stage 2: y_re = re1*Cr + im1*Ci ; y_im = im1*Cr - re1*Ci
                y = yp.tile([128, R * 256], F16, tag="y")
                yv_re = sview(y, 64, [[256, Rg], [32, 6], [1, 16]])
                yv_im = sview(y, 80, [[256, Rg], [32, 6], [1, 16]])
                nc.vector.tensor_tensor(full(t2), full(t1), full(Cr), MUL)
                nc.vector.tensor_tensor(full(t4), full(t3), full(Ci), MUL)
                nc.vector.tensor_tensor(yv_re, full(t2), full(t4), ADD)
                nc.vector.tensor_tensor(full(t2), full(t3), full(Cr), MUL)
                nc.vector.tensor_tensor(full(t4), full(t1), full(Ci), MUL)
                nc.vector.tensor_tensor(yv_im, full(t2), full(t4), SUB)
                # (0,n): y[16:64] = re0 * (re(n)^2 + im(n)^2), n=1..3
                s1 = tmpp.tile([128, R * 48], F16, tag="s1")
                s2 = tmpp.tile([128, R * 48], F16, tag="s2")
                nc.scalar.square(s1[0:WO, 0:Rg * 48], fmv(16, [[1, 48]]))
                nc.scalar.square(s2[0:WO, 0:Rg * 48],
                                 fmv(NC_RE + 16, [[1, 48]]))
                nc.vector.tensor_tensor(s1[0:WO, 0:Rg * 48],
                                        s1[0:WO, 0:Rg * 48],
                                        s2[0:WO, 0:Rg * 48], ADD)
                for j in range(3):   # replicate re0 x3 into s2
                    nc.vector.tensor_copy(
                        sview(s2, j * 16, [[48, Rg], [1, 16]]),
                        fmv(0, [[1, 16]]))
                nc.vector.tensor_tensor(
                    sview(y, 16, [[256, Rg], [1, 48]]),
                    s1[0:WO, 0:Rg * 48], s2[0:WO, 0:Rg * 48], MUL)
                # (0,0): y[0:16] = re0^3
                s3 = tmpp.tile([128, R * 16], F16, tag="s3")
                nc.scalar.square(s3[0:WO, 0:Rg * 16], fmv(0, [[1, 16]]))
                nc.vector.tensor_tensor(
                    sview(y, 0, [[256, Rg], [1, 16]]),
                    s3[0:WO, 0:Rg * 16], fmv(0, [[1, 16]]), MUL)
                # bias + relu + cast
                nc.vector.tensor_tensor(y[0:WO, 0:Rg * 256],
                                        y[0:WO, 0:Rg * 256],
                                        biasT[0:WO, 0:Rg * 256], ADD)
                nc.scalar.activation(y[0:WO, 0:Rg * 256],
                                     y[0:WO, 0:Rg * 256],
                                     mybir.ActivationFunctionType.Relu)
                # fp16 output DMA on the scalar HWDGE ring; host casts to f32
                nc.scalar.dma_start(
                    _ap(out_d, h0 * WO * 256,
                        [[256, WO], [WO * 256, Rg], [1, 256]]),
                    y[0:WO, 0:Rg * 256])

            # ---- main loop over input rows ----
            RB = 8      # input rows per stack-DMA block (kw-split DMAs)
            sA_t = {}
            sB_t = {}
            for r in range(H):
                if r % RB == 0:
                    sAb = stkp.tile([128, RB * WO], cdt, tag="sA",
                                    name=f"sA{r}")
                    sBb = stkp.tile([96, RB * WO], cdt, tag="sB",
                                    name=f"sB{r}")
                    for kw in range(4):
                        nc.sync.dma_start(
                            _ap(sAb[:], kw * 32 * (RB * WO),
                                [[RB * WO, 32], [WO, RB], [1, WO]]),
                            _ap(xt_d, r * C_IN * W + kw,
                                [[W, C_IN], [C_IN * W, RB], [1, WO]]))
                    for kw in range(3):
                        nc.sync.dma_start(
                            _ap(sBb[:], kw * 32 * (RB * WO),
                                [[RB * WO, 32], [WO, RB], [1, WO]]),
                            _ap(xt_d, r * C_IN * W + 4 + kw,
                                [[W, C_IN], [C_IN * W, RB], [1, WO]]))
                    sA_t = {r + j: (sAb, j) for j in range(RB)}
                    sB_t = {r + j: (sBb, j) for j in range(RB)}
                sA = sA_t[r][0][:, sA_t[r][1] * WO:(sA_t[r][1] + 1) * WO]
                sB = sB_t[r][0][:, sB_t[r][1] * WO:(sB_t[r][1] + 1) * WO]
                kh_lo = max(0, r - (HO - 1))
                kh_hi = min(KS - 1, r)
                for kh in range(kh_lo, kh_hi + 1):
                    h = r - kh
                    if kh == 0:
                        psum_by_h[h] = psp.tile([128, NPAD],
                                                mybir.dt.float32, tag="ps",
                                                name=f"ps{h}")
                    nc.tensor.matmul(psum_by_h[h][0:WO, :], sA,
                                     fA[:, kh * NPAD:(kh + 1) * NPAD],
                                     start=(kh == 0), stop=False)
                for kh in range(kh_lo, kh_hi + 1):
                    h = r - kh
                    nc.tensor.matmul(psum_by_h[h][0:WO, :], sB,
                                     fB[:, kh * NPAD:(kh + 1) * NPAD],
                                     start=False, stop=(kh == KS - 1))
                if r >= KS - 1:
                    h = r - (KS - 1)
                    j = h % R
                    if j == 0:
                        group["fm"] = fmp.tile([128, R * NCONV], F16,
                                               tag="fm", name=f"fm{h}")
                        group["h0"] = h
                        group["Rg"] = min(R, HO - h)
                    ps = psum_by_h.pop(h)
                    nc.scalar.copy(
                        group["fm"][0:WO, j * NCONV:(j + 1) * NCONV],
                        ps[0:WO, 0:NCONV])
                    if j == group["Rg"] - 1:
                        bisp(group["fm"], group["h0"], group["Rg"])
    nc.compile()
    return nc


def _get_program():
    global _PROGRAM
    if _PROGRAM is None:
        _PROGRAM = _build_program()
    return _PROGRAM


def _install_trace_shim():
    """antenv.axon_hooks is absent in this image; recreate via ctypes."""
    if "antenv.axon_hooks" in sys.modules:
        return
    try:
        from trn_agent_boot.trn_boot import _ntff_profile_via_ctypes
        hook = _ntff_profile_via_ctypes("/opt/axon/libaxon_pjrt.so")
    except Exception:
        hook = None
    m = types.ModuleType("antenv.axon_hooks")
    m.get_axon_ntff_profile_hook = lambda: hook
    m.set_axon_ntff_profile_hook = lambda h: None
    sys.modules["antenv.axon_hooks"] = m
    bass_utils.upload_artifacts = lambda tmpdir: tmpdir


def kernel(x, w, bias, _trace=False, _tmpdir=None):
    """Full inputs -> full output (8,122,122,256) float32."""
    x = np.asarray(x, dtype=np.float32)
    w = np.asarray(w, dtype=np.float32)
    bias = np.asarray(bias, dtype=np.float32)
    np_cdt = _np_conv_dtype()

    fd = _host_filters(w).astype(np_cdt)
    biasrep = np.broadcast_to(
        np.tile(bias.astype(np.float16), R)[None, :],
        (128, R * 256)).copy()
    in_maps = []
    for b in range(NB):
        xt = np.ascontiguousarray(x[b].transpose(0, 2, 1)).astype(np_cdt)
        in_maps.append({"xt": xt, "filt": fd, "biasrep": biasrep})

    nc = _get_program()
    kwargs = {}
    if _trace:
        _install_trace_shim()
        kwargs = dict(trace=True, tmpdir=_tmpdir)
    res = bass_utils.run_bass_kernel_spmd(nc, in_maps,
                                          core_ids=list(range(NB)), **kwargs)
    out = np.stack([res.results[b]["out"] for b in range(NB)],
                   axis=0).astype(np.float32)
    if _trace:
        return out, res
    return out


if __name__ == "__main__":
    d = np.load("/tmp/ref_io.npz")
    out = kernel(d["x"], d["w"], d["bias"])
    exp = d["expected"]
    rel = np.linalg.norm(out - exp) / np.linalg.norm(exp)
    print("rel_l2 =", rel)
